# revision 1
# baseline (speedup 1.0000x reference)
"""Cosine-similarity self-attention (Cos_Attn) on 8 Trainium2 NeuronCores.

Reference math (x: [C=512, W=64, H=64] fp32, N = W*H = 4096):
    q = x.reshape(C, N).T                  # [N, C]
    energy = q @ q.T                       # [N, N]
    cos    = energy / (|q_i| |q_j|)
    out    = softmax(cos, axis=-1)[None]   # [1, N, N]

Sharding: the N query rows are split across 8 cores (512 rows each). Every
core receives the full x (the keys) plus its own query column slice
xq = x[:, rows]; it computes its [512, N] slice of cos and the row softmax
locally; the host concatenates the 8 slices.

Per-core device pipeline (streamed in 512-column blocks):
  -  input x arrives in per-block DMAs spread over the three DMA-capable
     issue engines (SP + ACT hardware-DGE queues, GpSimd software-DGE) -
     a single queue measured only ~70-105 GB/s and paced the whole kernel.
  -  norms: squares (GPSIMD/DVE) -> ones-matmul column-sum (PE, bf16) into
     a 4-bank PSUM strip; rn = exp(-0.5*ln(ns)) on ACT in two 2048-wide
     chunks (Ln/Exp table-set switches cost ~1.3us each, so few wide chunks
     beat per-block ones).
  -  xn = x * rn into bf16 tiles (DVE; ko-broadcast of rn, a pattern
     validated on HW) - bf16 operands give full-rate PE + fast weight load.
  -  energy tiles = xnq^T @ xn (PE, bf16), K=4x128 accumulated in PSUM;
     softmax exp straight out of PSUM on ACT with accum_out row sums
     (max-subtraction skipped: cos is bounded in [-1, 1]).
  -  row scale 1/rowsum: per-partition scale on ACT (architectural
     free-affine operand) for half the row tiles, DVE multiply by a
     materialized [P,512] scale row (middle-dim stride-0 broadcast) for the
     other half; innermost-stride-0 APs and pointer-scalar TENSOR_SCALAR
     are avoided (measured wrong / 10x slow on HW).
"""

import numpy as np

_NCORES = 8
_P = 128

# set by the test harness only; the grading path keeps these defaults
TRACE = False
TRACE_CORES = None
LAST_RESULT = None

_built = None  # (nc, C, N)


def _build(C, N, RPC):
    """Build the single-NEFF Bass/Tile program.

    Inputs:  x [C, N] (all keys), xq [C, RPC] (this core's query columns).
    Output:  out [RPC, N] = softmax rows for this core's queries.
    """
    from contextlib import ExitStack

    import concourse.tile as tile
    from concourse import bacc, mybir

    f32 = mybir.dt.float32
    bf16 = mybir.dt.bfloat16
    AF = mybir.ActivationFunctionType
    AX = mybir.AxisListType
    OP = mybir.AluOpType

    P = _P
    KO = C // P              # contraction subtiles
    CB = 512                 # column block: one PSUM bank per energy tile
    NB = N // CB
    MT = RPC // P            # query row tiles per core
    HALF = max(KO // 2, 1)
    NH = KO // HALF          # square half-chunks per block
    RNC = min(NB, 4)         # rn chunk = RNC blocks (2048 cols)
    NRN = NB // RNC

    nc = bacc.Bacc("TRN2", target_bir_lowering=False, debug=False)
    x_d = nc.dram_tensor("x", [C, N], f32, kind="ExternalInput")
    xq_d = nc.dram_tensor("xq", [C, RPC], f32, kind="ExternalInput")
    out_d = nc.dram_tensor("out", [RPC, N], f32, kind="ExternalOutput")

    x_r = x_d.ap().rearrange("(ko p) n -> p ko n", p=P)
    xq_r = xq_d.ap().rearrange("(ko p) m -> p ko m", p=P)
    out_r = out_d.ap().rearrange("(mo p) n -> p mo n", p=P)

    with tile.TileContext(nc) as tc, ExitStack() as ctx:
        persist = ctx.enter_context(tc.tile_pool(name="persist", bufs=1))
        temps = ctx.enter_context(tc.tile_pool(name="temps", bufs=3))
        psum = ctx.enter_context(tc.tile_pool(name="psum", bufs=4, space="PSUM"))

        xn_sb = persist.tile([P, KO, N], bf16)     # normalized keys
        xnq_sb = persist.tile([P, KO, RPC], bf16)  # normalized queries
        rn = persist.tile([P, N], f32)             # 1/|q_n|, replicated on parts
        rnq = persist.tile([P, RPC], f32)
        e = persist.tile([P, MT, N], f32)          # exp(cos); scaled in place
        sums = persist.tile([P, MT, NB], f32)      # per-(m, nb) exp row sums
        rs = persist.tile([P, MT], f32)
        rr = persist.tile([P, MT], f32)
        ones = persist.tile([P, P], bf16)
        ones_row = persist.tile([P, CB], f32)
        nc.vector.memset(ones[:], 1.0)
        nc.vector.memset(ones_row[:], 1.0)

        # round-robin DMA issue over the three DMA-capable engines so the
        # transfers spread across independent DGE queues
        dma_engines = [nc.sync, nc.scalar, nc.gpsimd]
        dma_state = [0]

        def dma(out_ap, in_ap):
            eng = dma_engines[dma_state[0] % len(dma_engines)]
            dma_state[0] += 1
            eng.dma_start(out_ap, in_ap)

        sq_state = [0]

        def squares_and_colsum(src, width, ns_out):
            """ns_out [P, width] (PSUM) <- colsum over partitions of src^2."""
            for h in range(NH):
                xsq = temps.tile([P, HALF, width], bf16, tag="xsq", name="xsq",
                                 bufs=3)
                src_h = src[:, h * HALF:(h + 1) * HALF, :]
                # squares on GPSIMD (2 of 3) and DVE (1 of 3); ACT is the
                # busiest engine so it gets none
                if sq_state[0] % 3 == 2:
                    nc.vector.tensor_mul(xsq[:], src_h, src_h)
                else:
                    nc.gpsimd.tensor_mul(xsq[:], src_h, src_h)
                sq_state[0] += 1
                for k in range(HALF):
                    ko = h * HALF + k
                    nc.tensor.matmul(
                        ns_out,
                        lhsT=ones[:],
                        rhs=xsq[:, k, :],
                        start=(ko == 0),
                        stop=(ko == KO - 1),
                    )

        def rsqrt_act(dst, src_ps):
            """dst <- exp(-0.5*ln(src)); Ln/Exp live in one ACT table set."""
            nc.scalar.activation(dst, src_ps, AF.Ln)
            nc.scalar.activation(dst, dst, AF.Exp, scale=-0.5)

        def normalize(dst, src, rn_ap, width):
            """dst [P, KO, width] (bf16) <- src * rn (rn ko-broadcast)."""
            rn_b = rn_ap[:, None, :].to_broadcast([P, KO, width])
            nc.vector.tensor_mul(dst, src, rn_b)

        # ---- query-side prologue ----
        xqr = temps.tile([P, KO, RPC], f32, tag="xqr", name="xqr", bufs=1)
        dma(xqr[:], xq_r)
        nsq = psum.tile([P, RPC], f32, tag="ps", name="nsq")
        squares_and_colsum(xqr[:], RPC, nsq[:])
        rsqrt_act(rnq[:], nsq[:])
        normalize(xnq_sb[:], xqr[:], rnq[:], RPC)

        # ---- streamed main loop; rn computed per RNC-block chunk ----
        for rc in range(NRN):
            ns_strip = psum.tile([P, RNC, CB], f32, tag="nsb", name="ns_strip",
                                 bufs=1)
            xr_tiles = {}
            for j in range(RNC):
                nb = rc * RNC + j
                cs = slice(nb * CB, (nb + 1) * CB)
                xr = temps.tile([P, KO, CB], f32, tag="xr", name="xr", bufs=4)
                dma(xr[:, 0:HALF, :], x_r[:, 0:HALF, cs])
                dma(xr[:, HALF:KO, :], x_r[:, HALF:KO, cs])
                squares_and_colsum(xr[:], CB, ns_strip[:, j, :])
                xr_tiles[j] = xr  # raw block lives until normalize below
            ccs = slice(rc * RNC * CB, (rc + 1) * RNC * CB)
            rsqrt_act(rn[:, ccs], ns_strip[:].rearrange("p a b -> p (a b)"))
            # normalize + energy for the chunk's blocks
            for j in range(RNC):
                nb = rc * RNC + j
                cs = slice(nb * CB, (nb + 1) * CB)
                normalize(xn_sb[:, :, cs], xr_tiles[j][:], rn[:, cs], CB)
                for m in range(MT):
                    ms = slice(m * P, (m + 1) * P)
                    pt = psum.tile([P, CB], f32, tag="ps", name="pt")
                    for k in range(KO):
                        nc.tensor.matmul(
                            pt[:],
                            lhsT=xnq_sb[:, k, ms],
                            rhs=xn_sb[:, k, cs],
                            start=(k == 0),
                            stop=(k == KO - 1),
                        )
                    nc.scalar.activation(
                        e[:, m, cs], pt[:], AF.Exp,
                        accum_out=sums[:, m, nb:nb + 1],
                    )

        # ---- tail: row-normalize, stream out ----
        OUT_CHUNK = min(N, 2048)
        for m in range(MT):
            nc.vector.tensor_reduce(
                rs[:, m:m + 1], sums[:, m, :], axis=AX.X, op=OP.add
            )
            nc.vector.reciprocal(rr[:, m:m + 1], rs[:, m:m + 1])
            rr_m = rr[:, m:m + 1]
            if m % 2 == 1:
                # materialized scale row for the DVE middle-dim broadcast
                rr_row = temps.tile([P, CB], f32, tag="rr_row", name="rr_row",
                                    bufs=2)
                nc.scalar.activation(rr_row[:], ones_row[:], AF.Copy,
                                     scale=rr_m)
            for ci, c0 in enumerate(range(0, N, OUT_CHUNK)):
                ocs = slice(c0, c0 + OUT_CHUNK)
                if m % 2 == 0:
                    nc.scalar.activation(e[:, m, ocs], e[:, m, ocs], AF.Copy,
                                         scale=rr_m)
                else:
                    ev = e[:, m, ocs].rearrange("p (a b) -> p a b", b=CB)
                    rr_b = rr_row[:, None, :].to_broadcast(
                        [P, OUT_CHUNK // CB, CB])
                    nc.vector.tensor_mul(ev, ev, rr_b)
                dma(out_r[:, m, ocs], e[:, m, ocs])

    nc.compile()
    return nc


def kernel(**inputs) -> np.ndarray:
    global _built, LAST_RESULT
    x = np.ascontiguousarray(np.asarray(inputs["x"], dtype=np.float32))
    C, W, H = x.shape
    N = W * H
    RPC = N // _NCORES
    x2 = x.reshape(C, N)

    if _built is None or _built[1:] != (C, N):
        _built = (_build(C, N, RPC), C, N)
    nc = _built[0]

    from concourse import bass_utils

    in_maps = [
        {"x": x2, "xq": np.ascontiguousarray(x2[:, i * RPC:(i + 1) * RPC])}
        for i in range(_NCORES)
    ]
    kwargs = {}
    if TRACE:
        kwargs["trace"] = True
        if TRACE_CORES is not None:
            kwargs["trace_cores"] = list(TRACE_CORES)
    res = bass_utils.run_bass_kernel_spmd(
        nc, in_maps, core_ids=list(range(_NCORES)), **kwargs
    )
    LAST_RESULT = res
    out = np.concatenate([res.results[i]["out"] for i in range(_NCORES)], axis=0)
    return out.reshape(1, N, N)



# revision 4
# speedup vs baseline: 1.3090x; 1.3090x over previous
"""Cosine-similarity self-attention (Cos_Attn) on 8 Trainium2 NeuronCores.

Reference math (x: [C=512, W=64, H=64] fp32, N = W*H = 4096):
    q = x.reshape(C, N).T                  # [N, C]
    energy = q @ q.T                       # [N, N]
    cos    = energy / (|q_i| |q_j|)
    out    = softmax(cos, axis=-1)[None]   # [1, N, N]

Sharding: N query rows split across 8 cores (512 rows each). Queries are a
column subset of the keys, so each core gets the full x with its own 512
columns ROTATED to the front (host-side np.roll); the device then needs no
separate query input or query prologue - queries are block 0 of the
normalized keys. The host un-rotates the output columns per core.

v2 design (from the v1 trace: no engine >50% busy, ACT 54us with 9us of
table thrash, PE 46us with 20% on ones-colsums, 17MB DMA):
  -  x is cast to bf16 on host: input DMA halves to 4MB (matmuls were
     already bf16; norms from bf16 squares lose ~0.04%).
  -  output is bf16 (host casts back to f32): output DMA halves to 4MB.
     rel-err budget is 2e-2; bf16 out costs ~4e-3.
  -  energy in fp8e4 (e4m3) with DoubleRow perf mode: 0.5 cycles/row and
     K=256 per instruction -> 4x less PE streaming time than bf16.
     Both operands carry a c=16 range scale (folded into rn via the Exp
     bias) so fp8 sees ~N(0,0.7) values; exp() applies scale=1/c^2.
  -  softmax exp in [P,1024] groups (2 PSUM banks) with accum_out row
     sums; PSUM: ns strip 4 banks + 2 energy groups in flight = 8.
  -  rsqrt (Ln + Exp(-0.5)) on ACT in two 2048-wide chunks (table-set
     switches between natural_log and exp tables cost 1.28us each, so
     few wide chunks; all other ACT funcs stay in the exp table).
  -  tail: per-m rowsum finalize (DVE) -> rr_row materialize (ACT Copy,
     exp table) -> DVE middle-dim-broadcast multiply (validated fast
     pattern) -> out DMA per 2048 cols, pipelined across m tiles.
"""

import numpy as np

_NCORES = 8
_P = 128

# set by the test harness only; the grading path keeps these defaults
TRACE = False
TRACE_CORES = None
LAST_RESULT = None

_built = None  # (nc, C, N)

_C_SCALE = 16.0  # fp8 range scale on each operand; exp scale = 1/c^2


def _build(C, N, RPC):
    """Build the single-NEFF Bass/Tile program.

    Inputs:  x [C, N] bf16 (all keys, own block rotated to columns 0:RPC).
    Output:  out [RPC, N] bf16 = softmax rows for this core's queries,
             columns in the same rotated order.
    """
    import math
    from contextlib import ExitStack

    import concourse.tile as tile
    from concourse import bacc, mybir

    f32 = mybir.dt.float32
    bf16 = mybir.dt.bfloat16
    fp8 = mybir.dt.float8e4
    AF = mybir.ActivationFunctionType
    AX = mybir.AxisListType
    OP = mybir.AluOpType
    DR = mybir.MatmulPerfMode.DoubleRow

    P = _P
    KO = C // P              # contraction subtiles (4)
    CB = 512                 # column block: one PSUM bank per f32 [P,CB]
    NB = N // CB             # 8
    MT = RPC // P            # query row tiles per core (4)
    HALF_NB = 4              # blocks per rsqrt chunk (2048 cols)
    NHALF = NB // HALF_NB    # 2
    GW = 2                   # energy/exp group width in blocks (1024 cols)
    NG = N // (GW * CB)      # exp groups per m row-tile (4)
    c2inv = 1.0 / (_C_SCALE * _C_SCALE)
    lnc = math.log(_C_SCALE)

    nc = bacc.Bacc("TRN2", target_bir_lowering=False, debug=False)
    x_d = nc.dram_tensor("x", [C, N], bf16, kind="ExternalInput")
    out_d = nc.dram_tensor("out", [RPC, N], bf16, kind="ExternalOutput")

    x_r = x_d.ap().rearrange("(ko p) n -> p ko n", p=P)
    out_r = out_d.ap().rearrange("(mo p) n -> p mo n", p=P)

    with tile.TileContext(nc) as tc, ExitStack() as ctx:
        persist = ctx.enter_context(tc.tile_pool(name="persist", bufs=1))
        temps = ctx.enter_context(tc.tile_pool(name="temps", bufs=3))
        psum = ctx.enter_context(tc.tile_pool(name="psum", bufs=2, space="PSUM"))

        x_sb = persist.tile([P, KO, N], bf16)      # raw keys (whole input)
        xn8 = persist.tile([P, KO, N], fp8)        # c * normalized keys
        rn = persist.tile([P, N], f32)             # c / |q_n|, replicated
        lnt = persist.tile([P, HALF_NB * CB], f32) # ln(ns) scratch
        e = persist.tile([P, MT, N], bf16)         # exp(cos); scaled in place
        sums = persist.tile([P, MT, NG], f32)      # per-(m, group) row sums
        rs = persist.tile([P, MT], f32)
        rr = persist.tile([P, MT], f32)
        ones = persist.tile([P, P], bf16)
        ones_row = persist.tile([P, CB], f32)
        lnc_b = persist.tile([P, 1], f32)
        nc.vector.memset(ones[:], 1.0)
        nc.vector.memset(ones_row[:], 1.0)
        nc.vector.memset(lnc_b[:], lnc)

        # round-robin DMA issue over the three DMA-capable issue engines
        dma_engines = [nc.sync, nc.scalar, nc.gpsimd]
        dma_state = [0]

        def dma(out_ap, in_ap):
            eng = dma_engines[dma_state[0] % len(dma_engines)]
            dma_state[0] += 1
            eng.dma_start(out_ap, in_ap)

        sq_state = [0]

        def energy_group(m, g):
            """One [P, GW*CB] energy+exp group for row-tile m, group g."""
            ms = slice(m * P, (m + 1) * P)
            pt = psum.tile([P, GW, CB], f32, tag="pt", name="pt", bufs=2)
            for j in range(GW):
                nb = g * GW + j
                cs = slice(nb * CB, (nb + 1) * CB)
                for k in range(KO // 2):
                    ks = slice(2 * k, 2 * k + 2)
                    nc.tensor.matmul(
                        pt[:, j, :],
                        lhsT=xn8[:, ks, ms],
                        rhs=xn8[:, ks, cs],
                        start=(k == 0),
                        stop=(k == KO // 2 - 1),
                        perf_mode=DR,
                    )
            gs = slice(g * GW * CB, (g + 1) * GW * CB)
            nc.scalar.activation(
                e[:, m, gs], pt[:].rearrange("p a b -> p (a b)"), AF.Exp,
                scale=c2inv, accum_out=sums[:, m, g:g + 1],
            )

        # ---- streamed main loop over two halves of the key columns ----
        for h in range(NHALF):
            ns_ps = psum.tile([P, HALF_NB, CB], f32, tag="ns", name="ns",
                              bufs=1)
            for j in range(HALF_NB):
                nb = h * HALF_NB + j
                cs = slice(nb * CB, (nb + 1) * CB)
                dma(x_sb[:, 0:KO // 2, cs], x_r[:, 0:KO // 2, cs])
                dma(x_sb[:, KO // 2:KO, cs], x_r[:, KO // 2:KO, cs])
                xsq = temps.tile([P, KO, CB], bf16, tag="xsq", name="xsq",
                                 bufs=3)
                # squares split between DVE and GPSIMD
                if sq_state[0] % 2 == 0:
                    nc.vector.tensor_mul(xsq[:], x_sb[:, :, cs], x_sb[:, :, cs])
                else:
                    nc.gpsimd.tensor_mul(xsq[:], x_sb[:, :, cs], x_sb[:, :, cs])
                sq_state[0] += 1
                for k in range(KO):
                    nc.tensor.matmul(
                        ns_ps[:, j, :],
                        lhsT=ones[:],
                        rhs=xsq[:, k, :],
                        start=(k == 0),
                        stop=(k == KO - 1),
                    )
            # rsqrt chunk: rn = c * exp(-0.5*ln(ns))
            ccs = slice(h * HALF_NB * CB, (h + 1) * HALF_NB * CB)
            nc.scalar.activation(lnt[:], ns_ps[:].rearrange("p a b -> p (a b)"),
                                 AF.Ln)
            nc.scalar.activation(rn[:, ccs], lnt[:], AF.Exp, scale=-0.5,
                                 bias=lnc_b[:])
            # normalize + quantize to fp8 (per-ko 2D muls for DVE fast mode)
            for j in range(HALF_NB):
                nb = h * HALF_NB + j
                cs = slice(nb * CB, (nb + 1) * CB)
                for k in range(KO):
                    nc.vector.tensor_mul(xn8[:, k, cs], x_sb[:, k, cs],
                                         rn[:, cs])
            # energy + exp for this half's groups
            for m in range(MT):
                for gg in range(HALF_NB // GW):
                    energy_group(m, h * (HALF_NB // GW) + gg)

        # ---- tail: per-m row-normalize, stream out ----
        OUT_CHUNK = 2048
        for m in range(MT):
            nc.vector.tensor_reduce(
                rs[:, m:m + 1], sums[:, m, :], axis=AX.X, op=OP.add
            )
            nc.vector.reciprocal(rr[:, m:m + 1], rs[:, m:m + 1])
            rr_row = temps.tile([P, CB], f32, tag="rr_row", name="rr_row",
                                bufs=2)
            nc.scalar.activation(rr_row[:], ones_row[:], AF.Copy,
                                 scale=rr[:, m:m + 1])
            for c0 in range(0, N, OUT_CHUNK):
                ocs = slice(c0, c0 + OUT_CHUNK)
                ev = e[:, m, ocs].rearrange("p (a b) -> p a b", b=CB)
                rr_b = rr_row[:, None, :].to_broadcast(
                    [P, OUT_CHUNK // CB, CB])
                nc.vector.tensor_mul(ev, ev, rr_b)
                dma(out_r[:, m, ocs], e[:, m, ocs])

    nc.compile()
    return nc


def kernel(**inputs) -> np.ndarray:
    global _built, LAST_RESULT
    import ml_dtypes

    x = np.asarray(inputs["x"], dtype=np.float32)
    C, W, H = x.shape
    N = W * H
    RPC = N // _NCORES
    x2 = x.reshape(C, N)

    if _built is None or _built[1:] != (C, N):
        _built = (_build(C, N, RPC), C, N)
    nc = _built[0]

    from concourse import bass_utils

    in_maps = []
    for i in range(_NCORES):
        xi = np.roll(x2, -i * RPC, axis=1) if i else x2
        in_maps.append({"x": np.ascontiguousarray(xi.astype(ml_dtypes.bfloat16))})

    kwargs = {}
    if TRACE:
        kwargs["trace"] = True
        if TRACE_CORES is not None:
            kwargs["trace_cores"] = list(TRACE_CORES)
    res = bass_utils.run_bass_kernel_spmd(
        nc, in_maps, core_ids=list(range(_NCORES)), **kwargs
    )
    LAST_RESULT = res
    out = np.empty((N, N), dtype=np.float32)
    for i in range(_NCORES):
        blk = res.results[i]["out"].astype(np.float32)
        out[i * RPC:(i + 1) * RPC] = np.roll(blk, i * RPC, axis=1) if i else blk
    return out.reshape(1, N, N)


# revision 5
# speedup vs baseline: 1.4745x; 1.1264x over previous
"""Cosine-similarity self-attention (Cos_Attn) on 8 Trainium2 NeuronCores.

Reference math (x: [C=512, W=64, H=64] fp32, N = W*H = 4096):
    q = x.reshape(C, N).T                  # [N, C]
    energy = q @ q.T                       # [N, N]
    cos    = energy / (|q_i| |q_j|)
    out    = softmax(cos, axis=-1)[None]   # [1, N, N]

v3 design - transposed tiles, host-quantized fp8 keys. Rationale from the
v2 trace: DVE 1-byte (fp8) writes and 4-byte operands run at 1 el/cyc/lane
(fast modes need all-2-byte packed operands), so the 2.1M-element key
normalize was a 19us serial DVE chain; the replicated-layout rsqrt cost
7.9us of ACT plus table thrash.

Per core: compute the TRANSPOSED slice e^T[all 4096 keys, own 512 queries]:
  - keys arrive as fp8e4 (x * c_in, quantized on host, 2 MB DMA). The
    cosine is computed for the quantized vectors, so quantization only
    perturbs angles (~0.3% fro error), not lengths.
  - energy tile kt: out[key-part 128, query-free 512] = x8_kt^T @ xnq8
    (fp8 DoubleRow, K=256/instr, 0.5 cyc/row: 4x less PE time than bf16).
  - key norms: NOT via squares+colsum. Gram tiles G_kt = x8_kt^T x8_kt
    (PE) hold c_in^2*ns on the diagonal; extract via identity-mask
    multiply + reduce (DVE), then one tiny Ln/Exp pair on [P,32] gives
    scale_kt = rsqrt(diag)/c_q per PARTITION - applied for free as the
    exp() per-partition scale operand. No replicated rsqrt, no normalize
    of the 2.1M key elements.
  - queries: own 512 columns arrive bf16; squares (DVE 2x mode) ->
    ones-colsum (PE) -> Ln/Exp rsqrt -> quantize to fp8 (one block).
    ACT order Ln(q), Ln(k), Exp(q), Exp(k) keeps it to 2 table loads.
  - row softmax sums = colsum over key partitions: ones-matmul
    accumulation over all 32 e^T tiles into one PSUM bank (PE, free).
  - tail: reciprocal_approx_fast -> bf16 row vector; e^T tiles scaled by
    the replicated free-axis vector (all-bf16 DVE 2x mode), DMA out per
    1 MB chunk. Host transposes each core's [4096, 512] block.
"""

import numpy as np

_NCORES = 8
_P = 128

# set by the test harness only; the grading path keeps these defaults
TRACE = False
TRACE_CORES = None
LAST_RESULT = None

_built = None  # (nc, C, N)

_C_IN = 4.0    # host fp8 quantize scale for keys
_C_Q = 16.0    # device fp8 quantize scale for normalized queries


def _build(C, N, QB):
    """Single-NEFF Bass/Tile program.

    Inputs:  x8 [C, N] fp8e4 = c_in * x (all keys, host-quantized)
             xq [C, QB] bf16 (this core's raw query columns)
             idn [P, P] f32 identity (diag-extract mask)
    Output:  out [N, QB] bf16 = e^T slice (transposed softmax rows).
    """
    import math
    from contextlib import ExitStack

    import concourse.tile as tile
    from concourse import bacc, mybir

    f32 = mybir.dt.float32
    bf16 = mybir.dt.bfloat16
    fp8 = mybir.dt.float8e4
    AF = mybir.ActivationFunctionType
    AX = mybir.AxisListType
    OP = mybir.AluOpType
    DR = mybir.MatmulPerfMode.DoubleRow

    P = _P
    KO = C // P              # contraction subtiles (4)
    KT = N // P              # key tiles (32)
    lncq = math.log(_C_Q)

    nc = bacc.Bacc("TRN2", target_bir_lowering=False, debug=False)
    x8_d = nc.dram_tensor("x8", [C, N], fp8, kind="ExternalInput")
    xq_d = nc.dram_tensor("xq", [C, QB], bf16, kind="ExternalInput")
    idn_d = nc.dram_tensor("idn", [P, P], f32, kind="ExternalInput")
    out_d = nc.dram_tensor("out", [N, QB], bf16, kind="ExternalOutput")

    x8_r = x8_d.ap().rearrange("(ko p) n -> p ko n", p=P)
    xq_r = xq_d.ap().rearrange("(ko p) q -> p ko q", p=P)
    out_r = out_d.ap().rearrange("(kt p) q -> p kt q", p=P)

    with tile.TileContext(nc) as tc, ExitStack() as ctx:
        persist = ctx.enter_context(tc.tile_pool(name="persist", bufs=1))
        temps = ctx.enter_context(tc.tile_pool(name="temps", bufs=3))
        psum = ctx.enter_context(tc.tile_pool(name="psum", bufs=2, space="PSUM"))

        x8_sb = persist.tile([P, KO, N], fp8)      # raw fp8 keys
        xq_sb = persist.tile([P, KO, QB], bf16)    # raw bf16 queries
        xnq8 = persist.tile([P, KO, QB], fp8)      # c_q * normalized queries
        idn = persist.tile([P, P], f32)
        e_t = persist.tile([P, KT, QB], bf16)      # exp(cos)^T; scaled in place
        lnt = persist.tile([P, QB], f32)           # ln scratch (query side)
        rnq = persist.tile([P, QB], f32)           # c_q / |q| (replicated)
        nsd = persist.tile([P, KT], f32)           # key Gram diagonals
        scl = persist.tile([P, KT], f32)           # per-key exp scales
        rrf = persist.tile([P, QB], f32)
        rrb = persist.tile([P, QB], bf16)
        ones = persist.tile([P, P], bf16)
        lncq_b = persist.tile([P, 1], f32)
        nlncq_b = persist.tile([P, 1], f32)
        nc.vector.memset(ones[:], 1.0)
        nc.vector.memset(lncq_b[:], lncq)
        nc.vector.memset(nlncq_b[:], -lncq)

        dma_engines = [nc.sync, nc.scalar, nc.gpsimd]
        dma_state = [0]

        def dma(out_ap, in_ap):
            eng = dma_engines[dma_state[0] % len(dma_engines)]
            dma_state[0] += 1
            eng.dma_start(out_ap, in_ap)

        # ---- input DMAs (keys in 4 column-half chunks for early grams) ----
        dma(idn[:], idn_d.ap())
        dma(xq_sb[:, 0:2, :], xq_r[:, 0:2, :])
        dma(xq_sb[:, 2:4, :], xq_r[:, 2:4, :])
        H2 = N // 2
        for hc in range(2):
            cs = slice(hc * H2, (hc + 1) * H2)
            for kp in range(2):
                dma(x8_sb[:, 2 * kp:2 * kp + 2, cs],
                    x8_r[:, 2 * kp:2 * kp + 2, cs])

        # ---- query prologue: norms + fp8 quantize ----
        xsqq = temps.tile([P, KO, QB], bf16, tag="xsqq", name="xsqq", bufs=1)
        nc.vector.tensor_mul(xsqq[:], xq_sb[:], xq_sb[:])
        nsq = psum.tile([P, QB], f32, tag="nsq", name="nsq", bufs=1)
        for k in range(KO):
            nc.tensor.matmul(nsq[:], lhsT=ones[:], rhs=xsqq[:, k, :],
                             start=(k == 0), stop=(k == KO - 1))
        nc.scalar.activation(lnt[:], nsq[:], AF.Ln)

        # ---- key Gram diagonals ----
        for g in range(KT // 4):
            gps = psum.tile([P, 4, P], f32, tag="gram", name="gram", bufs=2)
            for t in range(4):
                kt = 4 * g + t
                ks = slice(kt * P, (kt + 1) * P)
                for k2 in range(KO // 2):
                    k2s = slice(2 * k2, 2 * k2 + 2)
                    nc.tensor.matmul(
                        gps[:, t, :],
                        lhsT=x8_sb[:, k2s, ks],
                        rhs=x8_sb[:, k2s, ks],
                        start=(k2 == 0),
                        stop=(k2 == KO // 2 - 1),
                        perf_mode=DR,
                    )
            mskd = temps.tile([P, 4, P], f32, tag="mskd", name="mskd", bufs=2)
            idn_b = idn[:, None, :].to_broadcast([P, 4, P])
            nc.vector.tensor_mul(mskd[:], gps[:], idn_b)
            nc.vector.tensor_reduce(nsd[:, 4 * g:4 * g + 4], mskd[:],
                                    axis=AX.X, op=OP.add)

        # ACT order Ln(q), Ln(k), Exp(q), Exp(k): 2 table loads total
        nc.scalar.activation(scl[:], nsd[:], AF.Ln)
        nc.scalar.activation(rnq[:], lnt[:], AF.Exp, scale=-0.5,
                             bias=lncq_b[:])
        nc.scalar.activation(scl[:], scl[:], AF.Exp, scale=-0.5,
                             bias=nlncq_b[:])

        # quantize queries (fp8 out is DVE slow-mode, but only one block)
        for k in range(KO):
            nc.vector.tensor_mul(xnq8[:, k, :], xq_sb[:, k, :], rnq[:])

        # ---- energy + exp + rowsum accumulation ----
        rs_ps = psum.tile([P, QB], f32, tag="rs", name="rs", bufs=1)
        for kt in range(KT):
            ks = slice(kt * P, (kt + 1) * P)
            pt = psum.tile([P, QB], f32, tag="pt", name="pt", bufs=4)
            for k2 in range(KO // 2):
                k2s = slice(2 * k2, 2 * k2 + 2)
                nc.tensor.matmul(
                    pt[:],
                    lhsT=x8_sb[:, k2s, ks],
                    rhs=xnq8[:, k2s, :],
                    start=(k2 == 0),
                    stop=(k2 == KO // 2 - 1),
                    perf_mode=DR,
                )
            nc.scalar.activation(e_t[:, kt, :], pt[:], AF.Exp,
                                 scale=scl[:, kt:kt + 1])
            nc.tensor.matmul(rs_ps[:], lhsT=ones[:], rhs=e_t[:, kt, :],
                             start=(kt == 0), stop=(kt == KT - 1))

        # ---- tail: row scale (free-axis, replicated) + out DMA ----
        nc.vector.reciprocal_approx_fast(rrf[:], rs_ps[:])
        nc.vector.tensor_scalar_mul(rrb[:], rrf[:], 1.0)
        CH = 8
        for h in range(KT // CH):
            hs = slice(h * CH, (h + 1) * CH)
            rr_b = rrb[:, None, :].to_broadcast([P, CH, QB])
            nc.vector.tensor_mul(e_t[:, hs, :], e_t[:, hs, :], rr_b)
            dma(out_r[:, hs, :], e_t[:, hs, :])

    nc.compile()
    return nc


def kernel(**inputs) -> np.ndarray:
    global _built, LAST_RESULT
    import ml_dtypes

    x = np.asarray(inputs["x"], dtype=np.float32)
    C, W, H = x.shape
    N = W * H
    QB = N // _NCORES
    x2 = x.reshape(C, N)

    if _built is None or _built[1:] != (C, N):
        _built = (_build(C, N, QB), C, N)
    nc = _built[0]

    from concourse import bass_utils

    x8 = np.ascontiguousarray((x2 * _C_IN).astype(ml_dtypes.float8_e4m3fn))
    idn = np.eye(_P, dtype=np.float32)
    in_maps = []
    for i in range(_NCORES):
        xq = np.ascontiguousarray(
            x2[:, i * QB:(i + 1) * QB].astype(ml_dtypes.bfloat16))
        in_maps.append({"x8": x8, "xq": xq, "idn": idn})

    kwargs = {}
    if TRACE:
        kwargs["trace"] = True
        if TRACE_CORES is not None:
            kwargs["trace_cores"] = list(TRACE_CORES)
    res = bass_utils.run_bass_kernel_spmd(
        nc, in_maps, core_ids=list(range(_NCORES)), **kwargs
    )
    LAST_RESULT = res
    out = np.empty((N, N), dtype=np.float32)
    for i in range(_NCORES):
        out[i * QB:(i + 1) * QB] = res.results[i]["out"].astype(np.float32).T
    return out.reshape(1, N, N)


# revision 6
# speedup vs baseline: 1.6025x; 1.0868x over previous
"""Cosine-similarity self-attention (Cos_Attn) on 8 Trainium2 NeuronCores.

Reference math (x: [C=512, W=64, H=64] fp32, N = W*H = 4096):
    q = x.reshape(C, N).T                  # [N, C]
    energy = q @ q.T                       # [N, N]
    cos    = energy / (|q_i| |q_j|)
    out    = softmax(cos, axis=-1)[None]   # [1, N, N]

v3 design - transposed tiles, host-quantized fp8 keys. Rationale from the
v2 trace: DVE 1-byte (fp8) writes and 4-byte operands run at 1 el/cyc/lane
(fast modes need all-2-byte packed operands), so the 2.1M-element key
normalize was a 19us serial DVE chain; the replicated-layout rsqrt cost
7.9us of ACT plus table thrash.

Per core: compute the TRANSPOSED slice e^T[all 4096 keys, own 512 queries]:
  - keys arrive as fp8e4 (x * c_in, quantized on host, 2 MB DMA). The
    cosine is computed for the quantized vectors, so quantization only
    perturbs angles (~0.3% fro error), not lengths.
  - energy tile kt: out[key-part 128, query-free 512] = x8_kt^T @ xnq8
    (fp8 DoubleRow, K=256/instr, 0.5 cyc/row: 4x less PE time than bf16).
  - key norms: NOT via squares+colsum. Gram tiles G_kt = x8_kt^T x8_kt
    (PE) hold c_in^2*ns on the diagonal; extract via identity-mask
    multiply + reduce (DVE), then one tiny Ln/Exp pair on [P,32] gives
    scale_kt = rsqrt(diag)/c_q per PARTITION - applied for free as the
    exp() per-partition scale operand. No replicated rsqrt, no normalize
    of the 2.1M key elements.
  - queries: own 512 columns arrive bf16; squares (DVE 2x mode) ->
    ones-colsum (PE) -> Ln/Exp rsqrt -> quantize to fp8 (one block).
    ACT order Ln(q), Ln(k), Exp(q), Exp(k) keeps it to 2 table loads.
  - row softmax sums = colsum over key partitions: ones-matmul
    accumulation over all 32 e^T tiles into one PSUM bank (PE, free).
  - tail: reciprocal_approx_fast -> bf16 row vector; e^T tiles scaled by
    the replicated free-axis vector (all-bf16 DVE 2x mode), DMA out per
    1 MB chunk. Host transposes each core's [4096, 512] block.
"""

import numpy as np

_NCORES = 8
_P = 128

# set by the test harness only; the grading path keeps these defaults
TRACE = False
TRACE_CORES = None
LAST_RESULT = None

_built = None  # (nc, C, N)

_C_IN = 4.0    # host fp8 quantize scale for keys
_C_Q = 16.0    # device fp8 quantize scale for normalized queries


def _build(C, N, QB):
    """Single-NEFF Bass/Tile program.

    Inputs:  x8 [C, N] fp8e4 = c_in * x (all keys, host-quantized)
             xq [C, QB] bf16 (this core's raw query columns)
             idn [P, P] f32 identity (diag-extract mask)
    Output:  out [N, QB] bf16 = e^T slice (transposed softmax rows).
    """
    import math
    from contextlib import ExitStack

    import concourse.tile as tile
    from concourse import bacc, mybir

    f32 = mybir.dt.float32
    bf16 = mybir.dt.bfloat16
    fp8 = mybir.dt.float8e4
    AF = mybir.ActivationFunctionType
    AX = mybir.AxisListType
    OP = mybir.AluOpType
    DR = mybir.MatmulPerfMode.DoubleRow

    P = _P
    KO = C // P              # contraction subtiles (4)
    KT = N // P              # key tiles (32)
    lncq = math.log(_C_Q)

    nc = bacc.Bacc("TRN2", target_bir_lowering=False, debug=False)
    x8_d = nc.dram_tensor("x8", [C, N], fp8, kind="ExternalInput")
    xq_d = nc.dram_tensor("xq", [C, QB], bf16, kind="ExternalInput")
    idn_d = nc.dram_tensor("idn", [P, P], f32, kind="ExternalInput")
    out_d = nc.dram_tensor("out", [N, QB], bf16, kind="ExternalOutput")

    x8_r = x8_d.ap().rearrange("(ko p) n -> p ko n", p=P)
    xq_r = xq_d.ap().rearrange("(ko p) q -> p ko q", p=P)
    out_r = out_d.ap().rearrange("(kt p) q -> p kt q", p=P)

    with tile.TileContext(nc) as tc, ExitStack() as ctx:
        persist = ctx.enter_context(tc.tile_pool(name="persist", bufs=1))
        temps = ctx.enter_context(tc.tile_pool(name="temps", bufs=3))
        psum = ctx.enter_context(tc.tile_pool(name="psum", bufs=2, space="PSUM"))

        x8_sb = persist.tile([P, KO, N], fp8)      # raw fp8 keys
        xq_sb = persist.tile([P, KO, QB], bf16)    # raw bf16 queries
        xnq8 = persist.tile([P, KO, QB], fp8)      # c_q * normalized queries
        idn = persist.tile([P, P], f32)
        e_t = persist.tile([P, KT, QB], bf16)      # exp(cos)^T; scaled in place
        lnt = persist.tile([P, QB], f32)           # ln scratch (query side)
        rnq = persist.tile([P, QB], f32)           # c_q / |q| (replicated)
        nsd = persist.tile([P, KT], f32)           # key Gram diagonals
        scl = persist.tile([P, KT], f32)           # per-key exp scales
        rrf = persist.tile([P, QB], f32)
        rrb = persist.tile([P, QB], bf16)
        ones = persist.tile([P, P], bf16)
        lncq_b = persist.tile([P, 1], f32)
        nlncq_b = persist.tile([P, 1], f32)
        nc.vector.memset(ones[:], 1.0)
        nc.vector.memset(lncq_b[:], lncq)
        nc.vector.memset(nlncq_b[:], -lncq)

        dma_engines = [nc.sync, nc.scalar]
        dma_state = [0]

        def dma(out_ap, in_ap):
            eng = dma_engines[dma_state[0] % len(dma_engines)]
            dma_state[0] += 1
            eng.dma_start(out_ap, in_ap)

        # ---- input DMAs: keys FIRST, in column quarters for early grams ----
        NQ4 = N // 4
        for qc in range(4):
            cs = slice(qc * NQ4, (qc + 1) * NQ4)
            for kp in range(2):
                dma(x8_sb[:, 2 * kp:2 * kp + 2, cs],
                    x8_r[:, 2 * kp:2 * kp + 2, cs])
        dma(xq_sb[:, 0:2, :], xq_r[:, 0:2, :])
        dma(xq_sb[:, 2:4, :], xq_r[:, 2:4, :])
        dma(idn[:], idn_d.ap())

        # ---- query prologue: norms + fp8 quantize ----
        xsqq = temps.tile([P, KO, QB], bf16, tag="xsqq", name="xsqq", bufs=1)
        nc.vector.tensor_mul(xsqq[:], xq_sb[:], xq_sb[:])
        nsq = psum.tile([P, QB], f32, tag="nsq", name="nsq", bufs=1)
        for k in range(KO):
            nc.tensor.matmul(nsq[:], lhsT=ones[:], rhs=xsqq[:, k, :],
                             start=(k == 0), stop=(k == KO - 1))
        nc.scalar.activation(lnt[:], nsq[:], AF.Ln)
        nc.scalar.activation(rnq[:], lnt[:], AF.Exp, scale=-0.5,
                             bias=lncq_b[:])

        # quantize queries (fp8 out is DVE slow-mode, but only one block)
        for k in range(KO):
            nc.vector.tensor_mul(xnq8[:, k, :], xq_sb[:, k, :], rnq[:])

        def grams(g):
            """Gram diagonals for key tiles 4g..4g+3 -> nsd."""
            gps = psum.tile([P, 4, P], f32, tag="gram", name="gram", bufs=2)
            for t in range(4):
                kt = 4 * g + t
                ks = slice(kt * P, (kt + 1) * P)
                for k2 in range(KO // 2):
                    k2s = slice(2 * k2, 2 * k2 + 2)
                    nc.tensor.matmul(
                        gps[:, t, :],
                        lhsT=x8_sb[:, k2s, ks],
                        rhs=x8_sb[:, k2s, ks],
                        start=(k2 == 0),
                        stop=(k2 == KO // 2 - 1),
                        perf_mode=DR,
                    )
            mskd = temps.tile([P, 4, P], f32, tag="mskd", name="mskd", bufs=2)
            idn_b = idn[:, None, :].to_broadcast([P, 4, P])
            nc.vector.tensor_mul(mskd[:], gps[:], idn_b)
            nc.vector.tensor_reduce(nsd[:, 4 * g:4 * g + 4], mskd[:],
                                    axis=AX.X, op=OP.add)

        def energy_exp(kt, rs_ps):
            ks = slice(kt * P, (kt + 1) * P)
            pt = psum.tile([P, QB], f32, tag="pt", name="pt", bufs=4)
            for k2 in range(KO // 2):
                k2s = slice(2 * k2, 2 * k2 + 2)
                nc.tensor.matmul(
                    pt[:],
                    lhsT=x8_sb[:, k2s, ks],
                    rhs=xnq8[:, k2s, :],
                    start=(k2 == 0),
                    stop=(k2 == KO // 2 - 1),
                    perf_mode=DR,
                )
            nc.scalar.activation(e_t[:, kt, :], pt[:], AF.Exp,
                                 scale=scl[:, kt:kt + 1])
            nc.tensor.matmul(rs_ps[:], lhsT=ones[:], rhs=e_t[:, kt, :],
                             start=(kt == 0), stop=(kt == KT - 1))

        # ---- per-half: grams -> scales -> energy/exp/rowsum ----
        rs_ps = psum.tile([P, QB], f32, tag="rs", name="rs", bufs=1)
        KTH = KT // 2
        for h in range(2):
            for g in range(4 * h, 4 * h + 4):
                grams(g)
            hs = slice(h * KTH, (h + 1) * KTH)
            nc.scalar.activation(scl[:, hs], nsd[:, hs], AF.Ln)
            nc.scalar.activation(scl[:, hs], scl[:, hs], AF.Exp, scale=-0.5,
                                 bias=nlncq_b[:])
            for kt in range(h * KTH, (h + 1) * KTH):
                energy_exp(kt, rs_ps)

        # ---- tail: row scale (free-axis, replicated) + out DMA ----
        nc.vector.reciprocal_approx_fast(rrf[:], rs_ps[:])
        nc.vector.tensor_scalar_mul(rrb[:], rrf[:], 1.0)
        CH = 8
        for h in range(KT // CH):
            hs = slice(h * CH, (h + 1) * CH)
            rr_b = rrb[:, None, :].to_broadcast([P, CH, QB])
            nc.vector.tensor_mul(e_t[:, hs, :], e_t[:, hs, :], rr_b)
            dma(out_r[:, hs, :], e_t[:, hs, :])

    nc.compile()
    return nc


def kernel(**inputs) -> np.ndarray:
    global _built, LAST_RESULT
    import ml_dtypes

    x = np.asarray(inputs["x"], dtype=np.float32)
    C, W, H = x.shape
    N = W * H
    QB = N // _NCORES
    x2 = x.reshape(C, N)

    if _built is None or _built[1:] != (C, N):
        _built = (_build(C, N, QB), C, N)
    nc = _built[0]

    from concourse import bass_utils

    x8 = np.ascontiguousarray((x2 * _C_IN).astype(ml_dtypes.float8_e4m3fn))
    idn = np.eye(_P, dtype=np.float32)
    in_maps = []
    for i in range(_NCORES):
        xq = np.ascontiguousarray(
            x2[:, i * QB:(i + 1) * QB].astype(ml_dtypes.bfloat16))
        in_maps.append({"x8": x8, "xq": xq, "idn": idn})

    kwargs = {}
    if TRACE:
        kwargs["trace"] = True
        if TRACE_CORES is not None:
            kwargs["trace_cores"] = list(TRACE_CORES)
    res = bass_utils.run_bass_kernel_spmd(
        nc, in_maps, core_ids=list(range(_NCORES)), **kwargs
    )
    LAST_RESULT = res
    out = np.empty((N, N), dtype=np.float32)
    for i in range(_NCORES):
        out[i * QB:(i + 1) * QB] = res.results[i]["out"].astype(np.float32).T
    return out.reshape(1, N, N)


# revision 9
# speedup vs baseline: 1.6544x; 1.0324x over previous
"""Cosine-similarity self-attention (Cos_Attn) on 8 Trainium2 NeuronCores.

Reference math (x: [C=512, W=64, H=64] fp32, N = W*H = 4096):
    q = x.reshape(C, N).T                  # [N, C]
    energy = q @ q.T                       # [N, N]
    cos    = energy / (|q_i| |q_j|)
    out    = softmax(cos, axis=-1)[None]   # [1, N, N]

v3 design - transposed tiles, host-quantized fp8 keys. Rationale from the
v2 trace: DVE 1-byte (fp8) writes and 4-byte operands run at 1 el/cyc/lane
(fast modes need all-2-byte packed operands), so the 2.1M-element key
normalize was a 19us serial DVE chain; the replicated-layout rsqrt cost
7.9us of ACT plus table thrash.

Per core: compute the TRANSPOSED slice e^T[all 4096 keys, own 512 queries]:
  - keys arrive as fp8e4 (x * c_in, quantized on host, 2 MB DMA). The
    cosine is computed for the quantized vectors, so quantization only
    perturbs angles (~0.3% fro error), not lengths.
  - energy tile kt: out[key-part 128, query-free 512] = x8_kt^T @ xnq8
    (fp8 DoubleRow, K=256/instr, 0.5 cyc/row: 4x less PE time than bf16).
  - key norms: NOT via squares+colsum. Gram tiles G_kt = x8_kt^T x8_kt
    (PE) hold c_in^2*ns on the diagonal; extract via identity-mask
    multiply + reduce (DVE), then one tiny Ln/Exp pair on [P,32] gives
    scale_kt = rsqrt(diag)/c_q per PARTITION - applied for free as the
    exp() per-partition scale operand. No replicated rsqrt, no normalize
    of the 2.1M key elements.
  - queries: own 512 columns arrive bf16; squares (DVE 2x mode) ->
    ones-colsum (PE) -> Ln/Exp rsqrt -> quantize to fp8 (one block).
    ACT order Ln(q), Ln(k), Exp(q), Exp(k) keeps it to 2 table loads.
  - row softmax sums = colsum over key partitions: ones-matmul
    accumulation over all 32 e^T tiles into one PSUM bank (PE, free).
  - tail: reciprocal_approx_fast -> bf16 row vector; e^T tiles scaled by
    the replicated free-axis vector (all-bf16 DVE 2x mode), DMA out per
    1 MB chunk. Host transposes each core's [4096, 512] block.
"""

import numpy as np

_NCORES = 8
_P = 128

# set by the test harness only; the grading path keeps these defaults
TRACE = False
TRACE_CORES = None
LAST_RESULT = None

_built = None  # (nc, C, N)

_C_IN = 4.0    # host fp8 quantize scale for keys
_C_Q = 16.0    # device fp8 quantize scale for normalized queries


def _build(C, N, QB):
    """Single-NEFF Bass/Tile program.

    Inputs:  x8 [C, N] fp8e4 = c_in * x (all keys, host-quantized)
             xq [C, QB] bf16 (this core's raw query columns)
             idn [P, P] f32 identity (diag-extract mask)
    Output:  out [N, QB] bf16 = e^T slice (transposed softmax rows).
    """
    import math
    from contextlib import ExitStack

    import concourse.tile as tile
    from concourse import bacc, mybir

    f32 = mybir.dt.float32
    bf16 = mybir.dt.bfloat16
    fp8 = mybir.dt.float8e4
    AF = mybir.ActivationFunctionType
    AX = mybir.AxisListType
    OP = mybir.AluOpType
    DR = mybir.MatmulPerfMode.DoubleRow

    P = _P
    KO = C // P              # contraction subtiles (4)
    KT = N // P              # key tiles (32)
    lncq = math.log(_C_Q)

    nc = bacc.Bacc("TRN2", target_bir_lowering=False, debug=False)
    x8_d = nc.dram_tensor("x8", [C, N], fp8, kind="ExternalInput")
    xq_d = nc.dram_tensor("xq", [C, QB], bf16, kind="ExternalInput")
    idn_d = nc.dram_tensor("idn", [P, P], f32, kind="ExternalInput")
    out_d = nc.dram_tensor("out", [N, QB], bf16, kind="ExternalOutput")

    x8_r = x8_d.ap().rearrange("(ko p) n -> p ko n", p=P)
    xq_r = xq_d.ap().rearrange("(ko p) q -> p ko q", p=P)
    out_r = out_d.ap().rearrange("(kt p) q -> p kt q", p=P)

    with tile.TileContext(nc) as tc, ExitStack() as ctx:
        persist = ctx.enter_context(tc.tile_pool(name="persist", bufs=1))
        temps = ctx.enter_context(tc.tile_pool(name="temps", bufs=3))
        psum = ctx.enter_context(tc.tile_pool(name="psum", bufs=2, space="PSUM"))

        x8_sb = persist.tile([P, KO, N], fp8)      # raw fp8 keys
        xq_sb = persist.tile([P, KO, QB], bf16)    # raw bf16 queries
        xnq8 = persist.tile([P, KO, QB], fp8)      # c_q * normalized queries
        idn = persist.tile([P, P], f32)
        e_t = persist.tile([P, KT, QB], bf16)      # exp(cos)^T; scaled in place
        lnt = persist.tile([P, QB], f32)           # ln scratch (query side)
        rnq = persist.tile([P, QB], f32)           # c_q / |q| (replicated)
        nsd = persist.tile([P, KT], f32)           # key Gram diagonals
        scl = persist.tile([P, KT], f32)           # per-key exp scales
        rrf = persist.tile([P, QB], f32)
        rrb = persist.tile([P, QB], bf16)
        ones = persist.tile([P, P], bf16)
        lncq_b = persist.tile([P, 1], f32)
        nlncq_b = persist.tile([P, 1], f32)
        nc.vector.memset(ones[:], 1.0)
        nc.vector.memset(lncq_b[:], lncq)
        nc.vector.memset(nlncq_b[:], -lncq)

        dma_engines = [nc.sync, nc.scalar]
        dma_state = [0]

        def dma(out_ap, in_ap):
            eng = dma_engines[dma_state[0] % len(dma_engines)]
            dma_state[0] += 1
            eng.dma_start(out_ap, in_ap)

        # ---- input DMAs: queries first (short prologue chain), then key
        # column quarters for early grams ----
        dma(xq_sb[:, 0:2, :], xq_r[:, 0:2, :])
        dma(xq_sb[:, 2:4, :], xq_r[:, 2:4, :])
        dma(idn[:], idn_d.ap())
        NQ4 = N // 4
        for qc in range(4):
            cs = slice(qc * NQ4, (qc + 1) * NQ4)
            for kp in range(2):
                dma(x8_sb[:, 2 * kp:2 * kp + 2, cs],
                    x8_r[:, 2 * kp:2 * kp + 2, cs])

        # ---- query prologue: norms ----
        xsqq = temps.tile([P, KO, QB], bf16, tag="xsqq", name="xsqq", bufs=1)
        nc.vector.tensor_mul(xsqq[:], xq_sb[:], xq_sb[:])
        nsq = psum.tile([P, QB], f32, tag="nsq", name="nsq", bufs=1)
        for k in range(KO):
            nc.tensor.matmul(nsq[:], lhsT=ones[:], rhs=xsqq[:, k, :],
                             start=(k == 0), stop=(k == KO - 1))
        nc.scalar.activation(lnt[:], nsq[:], AF.Ln)

        def grams(g):
            """Gram diagonals for key tiles 4g..4g+3 -> nsd."""
            gps = psum.tile([P, 4, P], f32, tag="gram", name="gram", bufs=2)
            for t in range(4):
                kt = 4 * g + t
                ks = slice(kt * P, (kt + 1) * P)
                for k2 in range(KO // 2):
                    k2s = slice(2 * k2, 2 * k2 + 2)
                    nc.tensor.matmul(
                        gps[:, t, :],
                        lhsT=x8_sb[:, k2s, ks],
                        rhs=x8_sb[:, k2s, ks],
                        start=(k2 == 0),
                        stop=(k2 == KO // 2 - 1),
                        perf_mode=DR,
                    )
            mskd = temps.tile([P, 4, P], f32, tag="mskd", name="mskd", bufs=2)
            idn_b = idn[:, None, :].to_broadcast([P, 4, P])
            nc.vector.tensor_mul(mskd[:], gps[:], idn_b)
            nc.vector.tensor_reduce(nsd[:, 4 * g:4 * g + 4], mskd[:],
                                    axis=AX.X, op=OP.add)

        def energy_exp(kt, rs_ps):
            ks = slice(kt * P, (kt + 1) * P)
            pt = psum.tile([P, QB], f32, tag="pt", name="pt", bufs=4)
            for k2 in range(KO // 2):
                k2s = slice(2 * k2, 2 * k2 + 2)
                nc.tensor.matmul(
                    pt[:],
                    lhsT=x8_sb[:, k2s, ks],
                    rhs=xnq8[:, k2s, :],
                    start=(k2 == 0),
                    stop=(k2 == KO // 2 - 1),
                    perf_mode=DR,
                )
            nc.scalar.activation(e_t[:, kt, :], pt[:], AF.Exp,
                                 scale=scl[:, kt:kt + 1])
            nc.tensor.matmul(rs_ps[:], lhsT=ones[:], rhs=e_t[:, kt, :],
                             start=(kt == 0), stop=(kt == KT - 1))

        # ---- grams, then one scale phase: ACT order Ln(q), Ln(k),
        # Exp(q), Exp(k) costs 2 table loads and no mid-chain cluster ----
        for g in range(KT // 4):
            grams(g)
        nc.scalar.activation(scl[:], nsd[:], AF.Ln)
        nc.scalar.activation(rnq[:], lnt[:], AF.Exp, scale=-0.5,
                             bias=lncq_b[:])
        nc.scalar.activation(scl[:], scl[:], AF.Exp, scale=-0.5,
                             bias=nlncq_b[:])

        # quantize queries (fp8 out is DVE slow-mode, but only one block)
        for k in range(KO):
            nc.vector.tensor_mul(xnq8[:, k, :], xq_sb[:, k, :], rnq[:])

        rs_ps = psum.tile([P, QB], f32, tag="rs", name="rs", bufs=1)
        for kt in range(KT):
            energy_exp(kt, rs_ps)

        # ---- tail: row scale (free-axis, replicated) + out DMA ----
        nc.vector.reciprocal_approx_fast(rrf[:], rs_ps[:])
        nc.vector.tensor_scalar_mul(rrb[:], rrf[:], 1.0)
        CH = 4
        for h in range(KT // CH):
            hs = slice(h * CH, (h + 1) * CH)
            rr_b = rrb[:, None, :].to_broadcast([P, CH, QB])
            nc.vector.tensor_mul(e_t[:, hs, :], e_t[:, hs, :], rr_b)
            dma(out_r[:, hs, :], e_t[:, hs, :])

    nc.compile()
    return nc


def kernel(**inputs) -> np.ndarray:
    global _built, LAST_RESULT
    import ml_dtypes

    x = np.asarray(inputs["x"], dtype=np.float32)
    C, W, H = x.shape
    N = W * H
    QB = N // _NCORES
    x2 = x.reshape(C, N)

    if _built is None or _built[1:] != (C, N):
        _built = (_build(C, N, QB), C, N)
    nc = _built[0]

    from concourse import bass_utils

    x8 = np.ascontiguousarray((x2 * _C_IN).astype(ml_dtypes.float8_e4m3fn))
    idn = np.eye(_P, dtype=np.float32)
    in_maps = []
    for i in range(_NCORES):
        xq = np.ascontiguousarray(
            x2[:, i * QB:(i + 1) * QB].astype(ml_dtypes.bfloat16))
        in_maps.append({"x8": x8, "xq": xq, "idn": idn})

    kwargs = {}
    if TRACE:
        kwargs["trace"] = True
        if TRACE_CORES is not None:
            kwargs["trace_cores"] = list(TRACE_CORES)
    res = bass_utils.run_bass_kernel_spmd(
        nc, in_maps, core_ids=list(range(_NCORES)), **kwargs
    )
    LAST_RESULT = res
    out = np.empty((N, N), dtype=np.float32)
    for i in range(_NCORES):
        out[i * QB:(i + 1) * QB] = res.results[i]["out"].astype(np.float32).T
    return out.reshape(1, N, N)


# revision 12
# speedup vs baseline: 1.6828x; 1.0172x over previous
"""Cosine-similarity self-attention (Cos_Attn) on 8 Trainium2 NeuronCores.

Reference math (x: [C=512, W=64, H=64] fp32, N = W*H = 4096):
    q = x.reshape(C, N).T                  # [N, C]
    energy = q @ q.T                       # [N, N]
    cos    = energy / (|q_i| |q_j|)
    out    = softmax(cos, axis=-1)[None]   # [1, N, N]

v3 design - transposed tiles, host-quantized fp8 keys. Rationale from the
v2 trace: DVE 1-byte (fp8) writes and 4-byte operands run at 1 el/cyc/lane
(fast modes need all-2-byte packed operands), so the 2.1M-element key
normalize was a 19us serial DVE chain; the replicated-layout rsqrt cost
7.9us of ACT plus table thrash.

Per core: compute the TRANSPOSED slice e^T[all 4096 keys, own 512 queries]:
  - keys arrive as fp8e4 (x * c_in, quantized on host, 2 MB DMA). The
    cosine is computed for the quantized vectors, so quantization only
    perturbs angles (~0.3% fro error), not lengths.
  - energy tile kt: out[key-part 128, query-free 512] = x8_kt^T @ xnq8
    (fp8 DoubleRow, K=256/instr, 0.5 cyc/row: 4x less PE time than bf16).
  - key norms: NOT via squares+colsum. Gram tiles G_kt = x8_kt^T x8_kt
    (PE) hold c_in^2*ns on the diagonal; extract via identity-mask
    multiply + reduce (DVE), then one tiny Ln/Exp pair on [P,32] gives
    scale_kt = rsqrt(diag)/c_q per PARTITION - applied for free as the
    exp() per-partition scale operand. No replicated rsqrt, no normalize
    of the 2.1M key elements.
  - queries: own 512 columns arrive bf16; squares (DVE 2x mode) ->
    ones-colsum (PE) -> Ln/Exp rsqrt -> quantize to fp8 (one block).
    ACT order Ln(q), Ln(k), Exp(q), Exp(k) keeps it to 2 table loads.
  - row softmax sums = colsum over key partitions: ones-matmul
    accumulation over all 32 e^T tiles into one PSUM bank (PE, free).
  - tail: reciprocal_approx_fast -> bf16 row vector; e^T tiles scaled by
    the replicated free-axis vector (all-bf16 DVE 2x mode), DMA out per
    1 MB chunk. Host transposes each core's [4096, 512] block.
"""

import numpy as np

_NCORES = 8
_P = 128

# set by the test harness only; the grading path keeps these defaults
TRACE = False
TRACE_CORES = None
LAST_RESULT = None

_built = None  # (nc, C, N)

_C_IN = 4.0    # host fp8 quantize scale for keys
_C_Q = 16.0    # device fp8 quantize scale for normalized queries


def _build(C, N, QB):
    """Single-NEFF Bass/Tile program.

    Inputs:  x8 [C, N] fp8e4 = c_in * x (all keys, host-quantized)
             xq [C, QB] bf16 (this core's raw query columns)
             idn [P, P] f32 identity (diag-extract mask)
    Output:  out [N, QB] bf16 = e^T slice (transposed softmax rows).
    """
    import math
    from contextlib import ExitStack

    import concourse.tile as tile
    from concourse import bacc, mybir

    f32 = mybir.dt.float32
    bf16 = mybir.dt.bfloat16
    fp8 = mybir.dt.float8e4
    AF = mybir.ActivationFunctionType
    AX = mybir.AxisListType
    OP = mybir.AluOpType
    DR = mybir.MatmulPerfMode.DoubleRow

    P = _P
    KO = C // P              # contraction subtiles (4)
    KT = N // P              # key tiles (32)
    lncq = math.log(_C_Q)

    nc = bacc.Bacc("TRN2", target_bir_lowering=False, debug=False)
    x8_d = nc.dram_tensor("x8", [C, N], fp8, kind="ExternalInput")
    xq_d = nc.dram_tensor("xq", [C, QB], bf16, kind="ExternalInput")
    idn_d = nc.dram_tensor("idn", [P, P], f32, kind="ExternalInput")
    out_d = nc.dram_tensor("out", [N, QB], bf16, kind="ExternalOutput")

    x8_r = x8_d.ap().rearrange("(ko p) n -> p ko n", p=P)
    xq_r = xq_d.ap().rearrange("(ko p) q -> p ko q", p=P)
    out_r = out_d.ap().rearrange("(kt p) q -> p kt q", p=P)

    with tile.TileContext(nc) as tc, ExitStack() as ctx:
        persist = ctx.enter_context(tc.tile_pool(name="persist", bufs=1))
        temps = ctx.enter_context(tc.tile_pool(name="temps", bufs=3))
        psum = ctx.enter_context(tc.tile_pool(name="psum", bufs=2, space="PSUM"))

        x8_sb = persist.tile([P, KO, N], fp8)      # raw fp8 keys
        xq_sb = persist.tile([P, KO, QB], bf16)    # raw bf16 queries
        xnq8 = persist.tile([P, KO, QB], fp8)      # c_q * normalized queries
        idn = persist.tile([P, P], f32)
        e_t = persist.tile([P, KT, QB], bf16)      # exp(cos)^T; scaled in place
        lnt = persist.tile([P, QB], f32)           # ln scratch (query side)
        rnq = persist.tile([P, QB], f32)           # c_q / |q| (replicated)
        nsd = persist.tile([P, KT], f32)           # key Gram diagonals
        scl = persist.tile([P, KT], f32)           # per-key exp scales
        rrf = persist.tile([P, QB], f32)
        rrb = persist.tile([P, QB], bf16)
        ones = persist.tile([P, P], bf16)
        lncq_b = persist.tile([P, 1], f32)
        nlncq_b = persist.tile([P, 1], f32)
        nc.vector.memset(ones[:], 1.0)
        nc.vector.memset(lncq_b[:], lncq)
        nc.vector.memset(nlncq_b[:], -lncq)

        dma_engines = [nc.sync, nc.scalar, nc.gpsimd]
        dma_state = [0]

        def dma(out_ap, in_ap):
            eng = dma_engines[dma_state[0] % len(dma_engines)]
            dma_state[0] += 1
            eng.dma_start(out_ap, in_ap)

        # ---- input DMAs: queries first (short prologue chain), then key
        # column quarters for early grams ----
        dma(xq_sb[:, 0:2, :], xq_r[:, 0:2, :])
        dma(xq_sb[:, 2:4, :], xq_r[:, 2:4, :])
        dma(idn[:], idn_d.ap())
        NQ4 = N // 4
        for qc in range(4):
            cs = slice(qc * NQ4, (qc + 1) * NQ4)
            for kp in range(2):
                dma(x8_sb[:, 2 * kp:2 * kp + 2, cs],
                    x8_r[:, 2 * kp:2 * kp + 2, cs])

        # ---- query prologue: norms ----
        xsqq = temps.tile([P, KO, QB], bf16, tag="xsqq", name="xsqq", bufs=1)
        nc.vector.tensor_mul(xsqq[:], xq_sb[:], xq_sb[:])
        nsq = psum.tile([P, QB], f32, tag="nsq", name="nsq", bufs=1)
        for k in range(KO):
            nc.tensor.matmul(nsq[:], lhsT=ones[:], rhs=xsqq[:, k, :],
                             start=(k == 0), stop=(k == KO - 1))
        nc.scalar.activation(lnt[:], nsq[:], AF.Ln)

        def grams(g):
            """Gram diagonals for key tiles 4g..4g+3 -> nsd."""
            gps = psum.tile([P, 4, P], f32, tag="gram", name="gram", bufs=2)
            for t in range(4):
                kt = 4 * g + t
                ks = slice(kt * P, (kt + 1) * P)
                for k2 in range(KO // 2):
                    k2s = slice(2 * k2, 2 * k2 + 2)
                    nc.tensor.matmul(
                        gps[:, t, :],
                        lhsT=x8_sb[:, k2s, ks],
                        rhs=x8_sb[:, k2s, ks],
                        start=(k2 == 0),
                        stop=(k2 == KO // 2 - 1),
                        perf_mode=DR,
                    )
            mskd = temps.tile([P, 4, P], f32, tag="mskd", name="mskd", bufs=2)
            idn_b = idn[:, None, :].to_broadcast([P, 4, P])
            nc.vector.tensor_mul(mskd[:], gps[:], idn_b)
            nc.vector.tensor_reduce(nsd[:, 4 * g:4 * g + 4], mskd[:],
                                    axis=AX.X, op=OP.add)

        def energy_exp(kt, rs_ps):
            ks = slice(kt * P, (kt + 1) * P)
            pt = psum.tile([P, QB], f32, tag="pt", name="pt", bufs=4)
            for k2 in range(KO // 2):
                k2s = slice(2 * k2, 2 * k2 + 2)
                nc.tensor.matmul(
                    pt[:],
                    lhsT=x8_sb[:, k2s, ks],
                    rhs=xnq8[:, k2s, :],
                    start=(k2 == 0),
                    stop=(k2 == KO // 2 - 1),
                    perf_mode=DR,
                )
            nc.scalar.activation(e_t[:, kt, :], pt[:], AF.Exp,
                                 scale=scl[:, kt:kt + 1])
            nc.tensor.matmul(rs_ps[:], lhsT=ones[:], rhs=e_t[:, kt, :],
                             start=(kt == 0), stop=(kt == KT - 1))

        # ---- per-half: grams -> scales -> exp chain. ACT issue order
        # Ln(q), Ln(k0), Exp(q), Exp(k0) costs 2 table loads up front; the
        # h1 cluster (Ln+Exp, 2 more loads) is a ~3.2us mid-chain hole but
        # h0's chain starts ~6us earlier than waiting for all grams ----
        rs_ps = psum.tile([P, QB], f32, tag="rs", name="rs", bufs=1)
        KTH = KT // 2
        for h in range(2):
            for g in range(4 * h, 4 * h + 4):
                grams(g)
            hs = slice(h * KTH, (h + 1) * KTH)
            nc.scalar.activation(scl[:, hs], nsd[:, hs], AF.Ln)
            if h == 0:
                nc.scalar.activation(rnq[:], lnt[:], AF.Exp, scale=-0.5,
                                     bias=lncq_b[:])
            nc.scalar.activation(scl[:, hs], scl[:, hs], AF.Exp, scale=-0.5,
                                 bias=nlncq_b[:])
            if h == 0:
                # quantize queries (fp8 out is DVE slow-mode; one block)
                for k in range(KO):
                    nc.vector.tensor_mul(xnq8[:, k, :], xq_sb[:, k, :],
                                         rnq[:])
            for kt in range(h * KTH, (h + 1) * KTH):
                energy_exp(kt, rs_ps)

        # ---- tail: row scale (free-axis, replicated) + out DMA ----
        nc.vector.reciprocal_approx_fast(rrf[:], rs_ps[:])
        nc.vector.tensor_scalar_mul(rrb[:], rrf[:], 1.0)
        CH = 2
        for h in range(KT // CH):
            hs = slice(h * CH, (h + 1) * CH)
            rr_b = rrb[:, None, :].to_broadcast([P, CH, QB])
            nc.vector.tensor_mul(e_t[:, hs, :], e_t[:, hs, :], rr_b)
            dma(out_r[:, hs, :], e_t[:, hs, :])

    nc.compile()
    return nc


def kernel(**inputs) -> np.ndarray:
    global _built, LAST_RESULT
    import ml_dtypes

    x = np.asarray(inputs["x"], dtype=np.float32)
    C, W, H = x.shape
    N = W * H
    QB = N // _NCORES
    x2 = x.reshape(C, N)

    if _built is None or _built[1:] != (C, N):
        _built = (_build(C, N, QB), C, N)
    nc = _built[0]

    from concourse import bass_utils

    x8 = np.ascontiguousarray((x2 * _C_IN).astype(ml_dtypes.float8_e4m3fn))
    idn = np.eye(_P, dtype=np.float32)
    in_maps = []
    for i in range(_NCORES):
        xq = np.ascontiguousarray(
            x2[:, i * QB:(i + 1) * QB].astype(ml_dtypes.bfloat16))
        in_maps.append({"x8": x8, "xq": xq, "idn": idn})

    kwargs = {}
    if TRACE:
        kwargs["trace"] = True
        if TRACE_CORES is not None:
            kwargs["trace_cores"] = list(TRACE_CORES)
    res = bass_utils.run_bass_kernel_spmd(
        nc, in_maps, core_ids=list(range(_NCORES)), **kwargs
    )
    LAST_RESULT = res
    out = np.empty((N, N), dtype=np.float32)
    for i in range(_NCORES):
        out[i * QB:(i + 1) * QB] = res.results[i]["out"].astype(np.float32).T
    return out.reshape(1, N, N)


# revision 13
# speedup vs baseline: 1.6955x; 1.0075x over previous
"""Cosine-similarity self-attention (Cos_Attn) on 8 Trainium2 NeuronCores.

Reference math (x: [C=512, W=64, H=64] fp32, N = W*H = 4096):
    q = x.reshape(C, N).T                  # [N, C]
    energy = q @ q.T                       # [N, N]
    cos    = energy / (|q_i| |q_j|)
    out    = softmax(cos, axis=-1)[None]   # [1, N, N]

v3 design - transposed tiles, host-quantized fp8 keys. Rationale from the
v2 trace: DVE 1-byte (fp8) writes and 4-byte operands run at 1 el/cyc/lane
(fast modes need all-2-byte packed operands), so the 2.1M-element key
normalize was a 19us serial DVE chain; the replicated-layout rsqrt cost
7.9us of ACT plus table thrash.

Per core: compute the TRANSPOSED slice e^T[all 4096 keys, own 512 queries]:
  - keys arrive as fp8e4 (x * c_in, quantized on host, 2 MB DMA). The
    cosine is computed for the quantized vectors, so quantization only
    perturbs angles (~0.3% fro error), not lengths.
  - energy tile kt: out[key-part 128, query-free 512] = x8_kt^T @ xnq8
    (fp8 DoubleRow, K=256/instr, 0.5 cyc/row: 4x less PE time than bf16).
  - key norms: NOT via squares+colsum. Gram tiles G_kt = x8_kt^T x8_kt
    (PE) hold c_in^2*ns on the diagonal; extract via identity-mask
    multiply + reduce (DVE), then one tiny Ln/Exp pair on [P,32] gives
    scale_kt = rsqrt(diag)/c_q per PARTITION - applied for free as the
    exp() per-partition scale operand. No replicated rsqrt, no normalize
    of the 2.1M key elements.
  - queries: own 512 columns arrive bf16; squares (DVE 2x mode) ->
    ones-colsum (PE) -> Ln/Exp rsqrt -> quantize to fp8 (one block).
    ACT order Ln(q), Ln(k), Exp(q), Exp(k) keeps it to 2 table loads.
  - row softmax sums = colsum over key partitions: ones-matmul
    accumulation over all 32 e^T tiles into one PSUM bank (PE, free).
  - tail: reciprocal_approx_fast -> bf16 row vector; e^T tiles scaled by
    the replicated free-axis vector (all-bf16 DVE 2x mode), DMA out per
    1 MB chunk. Host transposes each core's [4096, 512] block.
"""

import numpy as np

_NCORES = 8
_P = 128

# set by the test harness only; the grading path keeps these defaults
TRACE = False
TRACE_CORES = None
LAST_RESULT = None

_built = None  # (nc, C, N)

_C_IN = 4.0    # host fp8 quantize scale for keys
_C_Q = 16.0    # device fp8 quantize scale for normalized queries


def _build(C, N, QB):
    """Single-NEFF Bass/Tile program.

    Inputs:  x8 [C, N] fp8e4 = c_in * x (all keys, host-quantized)
             xq [C, QB] bf16 (this core's raw query columns)
             idn [P, P] f32 identity (diag-extract mask)
    Output:  out [N, QB] bf16 = e^T slice (transposed softmax rows).
    """
    import math
    from contextlib import ExitStack

    import concourse.tile as tile
    from concourse import bacc, mybir

    f32 = mybir.dt.float32
    bf16 = mybir.dt.bfloat16
    fp8 = mybir.dt.float8e4
    AF = mybir.ActivationFunctionType
    AX = mybir.AxisListType
    OP = mybir.AluOpType
    DR = mybir.MatmulPerfMode.DoubleRow

    P = _P
    KO = C // P              # contraction subtiles (4)
    KT = N // P              # key tiles (32)
    lncq = math.log(_C_Q)

    nc = bacc.Bacc("TRN2", target_bir_lowering=False, debug=False)
    x8_d = nc.dram_tensor("x8", [C, N], fp8, kind="ExternalInput")
    xq_d = nc.dram_tensor("xq", [C, QB], bf16, kind="ExternalInput")
    idn_d = nc.dram_tensor("idn", [P, P], f32, kind="ExternalInput")
    out_d = nc.dram_tensor("out", [N, QB], bf16, kind="ExternalOutput")

    x8_r = x8_d.ap().rearrange("(ko p) n -> p ko n", p=P)
    xq_r = xq_d.ap().rearrange("(ko p) q -> p ko q", p=P)
    out_r = out_d.ap().rearrange("(kt p) q -> p kt q", p=P)

    with tile.TileContext(nc) as tc, ExitStack() as ctx:
        persist = ctx.enter_context(tc.tile_pool(name="persist", bufs=1))
        temps = ctx.enter_context(tc.tile_pool(name="temps", bufs=3))
        psum = ctx.enter_context(tc.tile_pool(name="psum", bufs=2, space="PSUM"))

        x8_sb = persist.tile([P, KO, N], fp8)      # raw fp8 keys
        xq_sb = persist.tile([P, KO, QB], bf16)    # raw bf16 queries
        xnq8 = persist.tile([P, KO, QB], fp8)      # c_q * normalized queries
        idn = persist.tile([P, P], f32)
        e_t = persist.tile([P, KT, QB], bf16)      # exp(cos)^T; scaled in place
        lnt = persist.tile([P, QB], f32)           # ln scratch (query side)
        rnq = persist.tile([P, QB], f32)           # c_q / |q| (replicated)
        nsd = persist.tile([P, KT], f32)           # key Gram diagonals
        scl = persist.tile([P, KT], f32)           # per-key exp scales
        rrf = persist.tile([P, QB], f32)
        rrb = persist.tile([P, QB], bf16)
        ones = persist.tile([P, P], bf16)
        lncq_b = persist.tile([P, 1], f32)
        nlncq_b = persist.tile([P, 1], f32)
        nc.vector.memset(ones[:], 1.0)
        nc.vector.memset(lncq_b[:], lncq)
        nc.vector.memset(nlncq_b[:], -lncq)

        dma_engines = [nc.sync, nc.scalar, nc.gpsimd]
        dma_state = [0]

        def dma(out_ap, in_ap):
            eng = dma_engines[dma_state[0] % len(dma_engines)]
            dma_state[0] += 1
            eng.dma_start(out_ap, in_ap)

        # ---- input DMAs. Each issue engine owns ONE ~100 GB/s dynamic
        # queue (qSpDynamicHW / qActDynamicHW / qPoolDynamic), so spread
        # every tensor across all three engines and put xq (which gates
        # the whole query-prologue chain) strictly first on each queue ----
        for k in range(KO):
            dma(xq_sb[:, k:k + 1, :], xq_r[:, k:k + 1, :])
        dma(idn[:], idn_d.ap())
        NQ4 = N // 4
        for qc in range(4):
            cs = slice(qc * NQ4, (qc + 1) * NQ4)
            for kp in range(2):
                dma(x8_sb[:, 2 * kp:2 * kp + 2, cs],
                    x8_r[:, 2 * kp:2 * kp + 2, cs])

        # ---- query prologue: norms ----
        xsqq = temps.tile([P, KO, QB], bf16, tag="xsqq", name="xsqq", bufs=1)
        nc.vector.tensor_mul(xsqq[:], xq_sb[:], xq_sb[:])
        nsq = psum.tile([P, QB], f32, tag="nsq", name="nsq", bufs=1)
        for k in range(KO):
            nc.tensor.matmul(nsq[:], lhsT=ones[:], rhs=xsqq[:, k, :],
                             start=(k == 0), stop=(k == KO - 1))
        nc.scalar.activation(lnt[:], nsq[:], AF.Ln)

        def grams(g):
            """Gram diagonals for key tiles 4g..4g+3 -> nsd."""
            gps = psum.tile([P, 4, P], f32, tag="gram", name="gram", bufs=2)
            for t in range(4):
                kt = 4 * g + t
                ks = slice(kt * P, (kt + 1) * P)
                for k2 in range(KO // 2):
                    k2s = slice(2 * k2, 2 * k2 + 2)
                    nc.tensor.matmul(
                        gps[:, t, :],
                        lhsT=x8_sb[:, k2s, ks],
                        rhs=x8_sb[:, k2s, ks],
                        start=(k2 == 0),
                        stop=(k2 == KO // 2 - 1),
                        perf_mode=DR,
                    )
            mskd = temps.tile([P, 4, P], f32, tag="mskd", name="mskd", bufs=2)
            idn_b = idn[:, None, :].to_broadcast([P, 4, P])
            nc.vector.tensor_mul(mskd[:], gps[:], idn_b)
            nc.vector.tensor_reduce(nsd[:, 4 * g:4 * g + 4], mskd[:],
                                    axis=AX.X, op=OP.add)

        def energy_exp(kt, rs_ps):
            ks = slice(kt * P, (kt + 1) * P)
            pt = psum.tile([P, QB], f32, tag="pt", name="pt", bufs=4)
            for k2 in range(KO // 2):
                k2s = slice(2 * k2, 2 * k2 + 2)
                nc.tensor.matmul(
                    pt[:],
                    lhsT=x8_sb[:, k2s, ks],
                    rhs=xnq8[:, k2s, :],
                    start=(k2 == 0),
                    stop=(k2 == KO // 2 - 1),
                    perf_mode=DR,
                )
            nc.scalar.activation(e_t[:, kt, :], pt[:], AF.Exp,
                                 scale=scl[:, kt:kt + 1])
            nc.tensor.matmul(rs_ps[:], lhsT=ones[:], rhs=e_t[:, kt, :],
                             start=(kt == 0), stop=(kt == KT - 1))

        # ---- per-half: grams -> scales -> exp chain. ACT issue order
        # Ln(q), Ln(k0), Exp(q), Exp(k0) costs 2 table loads up front; the
        # h1 cluster (Ln+Exp, 2 more loads) is a ~3.2us mid-chain hole but
        # h0's chain starts ~6us earlier than waiting for all grams ----
        rs_ps = psum.tile([P, QB], f32, tag="rs", name="rs", bufs=1)
        KTH = KT // 2
        for h in range(2):
            for g in range(4 * h, 4 * h + 4):
                grams(g)
            hs = slice(h * KTH, (h + 1) * KTH)
            nc.scalar.activation(scl[:, hs], nsd[:, hs], AF.Ln)
            if h == 0:
                nc.scalar.activation(rnq[:], lnt[:], AF.Exp, scale=-0.5,
                                     bias=lncq_b[:])
            nc.scalar.activation(scl[:, hs], scl[:, hs], AF.Exp, scale=-0.5,
                                 bias=nlncq_b[:])
            if h == 0:
                # quantize queries (fp8 out is DVE slow-mode; one block)
                for k in range(KO):
                    nc.vector.tensor_mul(xnq8[:, k, :], xq_sb[:, k, :],
                                         rnq[:])
            for kt in range(h * KTH, (h + 1) * KTH):
                energy_exp(kt, rs_ps)

        # ---- tail: row scale (free-axis, replicated) + out DMA ----
        nc.vector.reciprocal_approx_fast(rrf[:], rs_ps[:])
        nc.vector.tensor_scalar_mul(rrb[:], rrf[:], 1.0)
        CH = 2
        for h in range(KT // CH):
            hs = slice(h * CH, (h + 1) * CH)
            rr_b = rrb[:, None, :].to_broadcast([P, CH, QB])
            nc.vector.tensor_mul(e_t[:, hs, :], e_t[:, hs, :], rr_b)
            dma(out_r[:, hs, :], e_t[:, hs, :])

    nc.compile()
    return nc


def kernel(**inputs) -> np.ndarray:
    global _built, LAST_RESULT
    import ml_dtypes

    x = np.asarray(inputs["x"], dtype=np.float32)
    C, W, H = x.shape
    N = W * H
    QB = N // _NCORES
    x2 = x.reshape(C, N)

    if _built is None or _built[1:] != (C, N):
        _built = (_build(C, N, QB), C, N)
    nc = _built[0]

    from concourse import bass_utils

    x8 = np.ascontiguousarray((x2 * _C_IN).astype(ml_dtypes.float8_e4m3fn))
    idn = np.eye(_P, dtype=np.float32)
    in_maps = []
    for i in range(_NCORES):
        xq = np.ascontiguousarray(
            x2[:, i * QB:(i + 1) * QB].astype(ml_dtypes.bfloat16))
        in_maps.append({"x8": x8, "xq": xq, "idn": idn})

    kwargs = {}
    if TRACE:
        kwargs["trace"] = True
        if TRACE_CORES is not None:
            kwargs["trace_cores"] = list(TRACE_CORES)
    res = bass_utils.run_bass_kernel_spmd(
        nc, in_maps, core_ids=list(range(_NCORES)), **kwargs
    )
    LAST_RESULT = res
    out = np.empty((N, N), dtype=np.float32)
    for i in range(_NCORES):
        out[i * QB:(i + 1) * QB] = res.results[i]["out"].astype(np.float32).T
    return out.reshape(1, N, N)
